# revision 1
# baseline (speedup 1.0000x reference)
"""Dilated attention Trainium2 kernel (8-core SPMD, sequence-sharded).

Sequence is split into 8 contiguous chunks of 512 rows (+24-row halo each
side).  Dilation=8 decomposes positions into 8 residue classes mod 8; within
a class, attention is a +/-3 banded window.  Per core:

  hT   [512,560]  = W_in^T @ x_halo^T (+b_in)            (f32r matmuls)
  tseT [1536,560] = W_tse^T @ hT                          (f32r, evicted f16)
  per (class c, head h):  scoresT[70k,64q] = maskT + kT.T qT   (f16 matmuls)
      attn = exp(scoresT/8) ; out[64q,64hd] = attn.T [v|1]     (f16)
  out_cm [512 s',512] (class-major row order) -> PE transpose -> out_cmT
  y [512,512] = out_cm @ W_out + b_out                    (f32r)
  final DMA scatters class-major rows back to sequence order.
"""
import sys
sys.path.insert(0, "/opt/trn_rl_repo")

from contextlib import ExitStack

import numpy as np

import concourse.bass as bass
import concourse.mybir as mybir
import concourse.tile as tile
from concourse import bacc
from concourse.bass_utils import run_bass_kernel_spmd

F32 = mybir.dt.float32
F32R = mybir.dt.float32r
F16 = mybir.dt.float16

N_CORES = 8
S = 4096
IN_DIM = 768
DIM = 512
H = 8
HD = 64
CHUNK = 512          # output rows per core
HALO = 24            # 3 * dilation
LOC = CHUNK + 2 * HALO   # 560 local rows
CL = LOC // 8        # 70 class positions per class
QL = CHUNK // 8      # 64 query positions per class
MASKVAL = -60000.0


def emit_body(tc, d, rep, debug=False):
    """d: dict of dram tensor handles."""
    nc = tc.nc
    ctx = ExitStack()
    with ctx:
        wp = ctx.enter_context(tc.tile_pool(name=f"w{rep}", bufs=1))

        # ---- load inputs ----
        xT = []
        for kt in range(6):
            t = wp.tile([128, LOC], F32, tag=f"xT{kt}")
            nc.sync.dma_start(t[:], d["xT"].ap()[kt * 128:(kt + 1) * 128, :])
            xT.append(t)
        W_in = []
        for kt in range(6):
            t = wp.tile([128, DIM], F32, tag=f"win{kt}")
            nc.sync.dma_start(t[:], d["W_in"].ap()[kt * 128:(kt + 1) * 128, :])
            W_in.append(t)
        W_tse = []
        for kt in range(4):
            t = wp.tile([128, 3 * DIM], F32, tag=f"wtse{kt}")
            nc.sync.dma_start(t[:], d["W_tse"].ap()[kt * 128:(kt + 1) * 128, :])
            W_tse.append(t)
        W_out = []
        for kt in range(4):
            t = wp.tile([128, DIM], F32, tag=f"wout{kt}")
            nc.sync.dma_start(t[:], d["W_out"].ap()[kt * 128:(kt + 1) * 128, :])
            W_out.append(t)
        b_in = wp.tile([128, 4], F32, tag="b_in")
        nc.sync.dma_start(b_in[:], d["b_in4"].ap().rearrange("c p one -> p (c one)"))
        b_out = wp.tile([1, DIM], F32, tag="b_out")
        nc.sync.dma_start(b_out[:], d["b_out_row"].ap())
        ones128 = wp.tile([1, 128], F32, tag="ones128")
        nc.sync.dma_start(ones128[:], d["ones128"].ap())
        maskT = wp.tile([QL, CL], F16, tag="maskT")
        nc.sync.dma_start(maskT[:], d["maskT"].ap())
        ones70 = wp.tile([CL, 1], F16, tag="ones70")
        nc.sync.dma_start(ones70[:], d["ones70"].ap())
        ident16 = wp.tile([128, 128], F16, tag="ident16")
        nc.sync.dma_start(ident16[:], d["ident16"].ap())
        ident32 = wp.tile([128, 128], F32, tag="ident32")
        nc.sync.dma_start(ident32[:], d["ident32"].ap())
        zbias = wp.tile([128, 1], F32, tag="zbias")
        nc.vector.memset(zbias[:], 0.0)

        # ---- round f32 -> f32r ----
        xTr = []
        for kt in range(6):
            t = wp.tile([128, LOC], F32R, tag=f"xTr{kt}")
            nc.vector.tensor_copy(t[:], xT[kt][:])
            xTr.append(t)
        W_inr = []
        for kt in range(6):
            t = wp.tile([128, DIM], F32R, tag=f"winr{kt}")
            nc.gpsimd.tensor_copy(t[:], W_in[kt][:])
            W_inr.append(t)
        W_tser = []
        for kt in range(4):
            t = wp.tile([128, 3 * DIM], F32R, tag=f"wtser{kt}")
            nc.gpsimd.tensor_copy(t[:], W_tse[kt][:])
            W_tser.append(t)
        W_outr = []
        for kt in range(4):
            t = wp.tile([128, DIM], F32R, tag=f"woutr{kt}")
            nc.gpsimd.tensor_copy(t[:], W_out[kt][:])
            W_outr.append(t)
        b_outr = wp.tile([1, DIM], F32R, tag="b_outr")
        nc.vector.tensor_copy(b_outr[:], b_out[:])
        ones128r = wp.tile([1, 128], F32R, tag="ones128r")
        nc.vector.tensor_copy(ones128r[:], ones128[:])

        HN = LOC // 2  # 280

        # ---- P1: hT = W_in^T @ xT (+ b_in) ----
        hTr = [wp.tile([128, LOC], F32R, tag=f"hTr{m}") for m in range(4)]
        with tc.tile_pool(name=f"psA{rep}", bufs=4, space="PSUM") as psA:
            for mc in range(4):
                for half in range(2):
                    ps = psA.tile([128, HN], F32, tag="ph")
                    for kt in range(6):
                        nc.tensor.matmul(
                            ps[:],
                            W_inr[kt][:, mc * 128:(mc + 1) * 128],
                            xTr[kt][:, half * HN:(half + 1) * HN],
                            start=(kt == 0), stop=(kt == 5),
                        )
                    nc.scalar.activation(
                        hTr[mc][:, half * HN:(half + 1) * HN], ps[:],
                        mybir.ActivationFunctionType.Identity,
                        bias=b_in[:, mc:mc + 1], scale=1.0,
                    )

            # ---- P2: tseT = W_tse^T @ hT  (f16 out) ----
            tseT = [wp.tile([128, LOC], F16, tag=f"tseT{c}") for c in range(12)]
            for ct in range(12):
                for half in range(2):
                    ps = psA.tile([128, HN], F32, tag="ph")
                    for kt in range(4):
                        nc.tensor.matmul(
                            ps[:],
                            W_tser[kt][:, ct * 128:(ct + 1) * 128],
                            hTr[kt][:, half * HN:(half + 1) * HN],
                            start=(kt == 0), stop=(kt == 3),
                        )
                    dst = tseT[ct][:, half * HN:(half + 1) * HN]
                    if ct % 2 == 0:
                        nc.scalar.copy(dst, ps[:])
                    else:
                        nc.vector.tensor_copy(dst, ps[:])

        # ---- P3: vt[c] = per-class transposed V ----
        vt = [wp.tile([CL, DIM], F16, tag=f"vt{c}") for c in range(8)]
        with tc.tile_pool(name=f"psT{rep}", bufs=4, space="PSUM") as psT:
            for c in range(8):
                for hp in range(4):
                    ps = psT.tile([CL, 128], F16, tag="pt")
                    nc.tensor.transpose(ps[:], tseT[8 + hp][:, c::8], ident16[:])
                    dst = vt[c][:, hp * 128:(hp + 1) * 128]
                    if hp % 2 == 0:
                        nc.scalar.copy(dst, ps[:])
                    else:
                        nc.vector.tensor_copy(dst, ps[:])

        # ---- P4: attention per (class, head) ----
        out_cm = [wp.tile([128, DIM], F32, tag=f"ocm{i}") for i in range(4)]
        ap_att = ctx.enter_context(tc.tile_pool(name=f"att{rep}", bufs=6))
        with (
            tc.tile_pool(name=f"ps1{rep}", bufs=4, space="PSUM") as ps1p,
            tc.tile_pool(name=f"ps2{rep}", bufs=4, space="PSUM") as ps2p,
        ):
            for c in range(8):
                for h in range(H):
                    hp, off = h // 2, (h % 2) * 64
                    ps1 = ps1p.tile([CL, QL], F32, tag="s")
                    # scoresT = mask + kT.T @ qT
                    nc.tensor.matmul(ps1[:], maskT[:], ident16[:QL, :QL],
                                     start=True, stop=False)
                    nc.tensor.matmul(
                        ps1[:],
                        tseT[4 + hp][off:off + 64, c::8],
                        tseT[hp][off:off + 64, HALO + c:HALO + c + 8 * QL:8],
                        start=False, stop=True,
                    )
                    attn = ap_att.tile([CL, QL], F16, tag="attn")
                    nc.scalar.activation(attn[:], ps1[:],
                                         mybir.ActivationFunctionType.Exp,
                                         bias=zbias[:CL, :], scale=0.125)
                    ps2 = ps2p.tile([QL, 65], F32, tag="o")
                    nc.tensor.matmul(ps2[:, 0:64], attn[:],
                                     vt[c][:, hp * 128 + off:hp * 128 + off + 64],
                                     start=True, stop=True)
                    nc.tensor.matmul(ps2[:, 64:65], attn[:], ones70[:],
                                     start=True, stop=True)
                    rec = ap_att.tile([QL, 1], F32, tag="rec")
                    nc.vector.reciprocal(rec[:], ps2[:, 64:65])
                    nc.vector.tensor_scalar_mul(
                        out_cm[c // 2][(c % 2) * 64:(c % 2) * 64 + 64,
                                       h * 64:(h + 1) * 64],
                        ps2[:, 0:64], rec[:],
                    )

        # ---- P5: transpose out_cm -> out_cmT (f32r) ----
        out_cmT = [wp.tile([128, DIM], F32R, tag=f"ocmT{j}") for j in range(4)]
        with tc.tile_pool(name=f"psO{rep}", bufs=4, space="PSUM") as psO:
            for j in range(4):
                for i in range(4):
                    ps = psO.tile([128, 128], F32, tag="po")
                    nc.tensor.transpose(ps[:], out_cm[i][:, j * 128:(j + 1) * 128],
                                        ident32[:])
                    dst = out_cmT[j][:, i * 128:(i + 1) * 128]
                    if i % 2 == 0:
                        nc.scalar.copy(dst, ps[:])
                    else:
                        nc.vector.tensor_copy(dst, ps[:])

        # ---- P6: y = out_cm @ W_out + b_out ----
        yp = ctx.enter_context(tc.tile_pool(name=f"y{rep}", bufs=2))
        with tc.tile_pool(name=f"psY{rep}", bufs=2, space="PSUM") as psY:
            for i in range(4):
                ps = psY.tile([128, DIM], F32, tag="py")
                for kt in range(4):
                    nc.tensor.matmul(ps[:],
                                     out_cmT[kt][:, i * 128:(i + 1) * 128],
                                     W_outr[kt][:],
                                     start=(kt == 0), stop=False)
                nc.tensor.matmul(ps[:], ones128r[:], b_outr[:],
                                 start=False, stop=True)
                ysb = yp.tile([128, DIM], F32, tag="ysb")
                nc.scalar.copy(ysb[:], ps[:])
                # class-major rows (c, t) -> sequence rows 8t + c
                yv = d["y"].ap().rearrange("(t c) n -> c t n", c=8)
                for cl in range(2):
                    nc.sync.dma_start(yv[2 * i + cl],
                                      ysb[cl * 64:(cl + 1) * 64, :])


_CACHE = {}


def build_program(repeats=1):
    if repeats in _CACHE:
        return _CACHE[repeats]
    nc = bacc.Bacc("TRN2", target_bir_lowering=False, debug=False,
                   num_devices=N_CORES)
    d = {}
    d["xT"] = nc.dram_tensor("xT", [IN_DIM, LOC], F32, kind="ExternalInput")
    d["W_in"] = nc.dram_tensor("W_in", [IN_DIM, DIM], F32, kind="ExternalInput")
    d["b_in4"] = nc.dram_tensor("b_in4", [4, 128, 1], F32, kind="ExternalInput")
    d["W_tse"] = nc.dram_tensor("W_tse", [DIM, 3 * DIM], F32, kind="ExternalInput")
    d["W_out"] = nc.dram_tensor("W_out", [DIM, DIM], F32, kind="ExternalInput")
    d["b_out_row"] = nc.dram_tensor("b_out_row", [1, DIM], F32, kind="ExternalInput")
    d["ones128"] = nc.dram_tensor("ones128", [1, 128], F32, kind="ExternalInput")
    d["maskT"] = nc.dram_tensor("maskT", [QL, CL], F16, kind="ExternalInput")
    d["ones70"] = nc.dram_tensor("ones70", [CL, 1], F16, kind="ExternalInput")
    d["ident16"] = nc.dram_tensor("ident16", [128, 128], F16, kind="ExternalInput")
    d["ident32"] = nc.dram_tensor("ident32", [128, 128], F32, kind="ExternalInput")
    d["y"] = nc.dram_tensor("y", [CHUNK, DIM], F32, kind="ExternalOutput")

    with tile.TileContext(nc) as tc:
        for rep in range(repeats):
            if rep > 0:
                tc.strict_bb_all_engine_barrier()
            emit_body(tc, d, rep)
    nc.compile()
    _CACHE[repeats] = nc
    return nc


def make_in_maps(x, W_in, b_in, W_tse, W_out, b_out):
    x = np.asarray(x, np.float32)
    xp = np.zeros((S + 2 * HALO, IN_DIM), np.float32)
    xp[HALO:HALO + S] = x

    shared = {
        "W_in": np.ascontiguousarray(W_in, np.float32),
        "b_in4": np.ascontiguousarray(b_in, np.float32).reshape(4, 128, 1),
        "W_tse": np.ascontiguousarray(W_tse, np.float32),
        "W_out": np.ascontiguousarray(W_out, np.float32),
        "b_out_row": np.ascontiguousarray(b_out, np.float32).reshape(1, DIM),
        "ones128": np.ones((1, 128), np.float32),
        "ones70": np.ones((CL, 1), np.float16),
        "ident16": np.eye(128, dtype=np.float16),
        "ident32": np.eye(128, dtype=np.float32),
    }

    tk = np.arange(CL)[:, None]          # key class-positions  [70,1]
    tq = np.arange(QL)[None, :] + 3      # query class-positions [1,64]
    band = np.abs(tk - tq) <= 3          # [70, 64]

    in_maps = []
    for r in range(N_CORES):
        xT_l = np.ascontiguousarray(xp[r * CHUNK: r * CHUNK + LOC].T)
        base = r * CHUNK - HALO
        kpos = base + 8 * tk             # global pos of (t_k, c=0); validity
        valid = (kpos >= 0) & (kpos + 7 < S)   # class-independent (see notes)
        m = np.where(band & valid, 0.0, MASKVAL).astype(np.float16)  # [70,64]
        maskT_l = np.ascontiguousarray(m.T)  # [64, 70]
        in_maps.append({"xT": xT_l, "maskT": maskT_l, **shared})
    return in_maps


def kernel(**inputs):
    x = inputs["x"]; W_in = inputs["W_in"]; b_in = inputs["b_in"]
    W_tse = inputs["W_tse"]; W_out = inputs["W_out"]; b_out = inputs["b_out"]
    nc = build_program(1)
    in_maps = make_in_maps(x, W_in, b_in, W_tse, W_out, b_out)
    res = run_bass_kernel_spmd(nc, in_maps, list(range(N_CORES)), trace=False)
    y = np.empty((S, DIM), np.float32)
    for r in range(N_CORES):
        y[r * CHUNK:(r + 1) * CHUNK] = res.results[r]["y"]
    return y
